# revision 5
# baseline (speedup 1.0000x reference)
"""nn_DEC_90125593739499 — Bass/Trainium2 kernel.

2x 2-layer GRU decoder with growing-context Bahdanau attention over the
per-layer hidden history, T=128 sequential steps.

Sharding: 8 cores = 2 decoder stacks x 4 batch quarters (b=8 rows/core);
weights replicated. Each core runs the full T=128 recurrence for one
stack on its batch shard; the final output projection (tiny) runs on
host. All cores execute the same program (SPMD) with per-core data.

On-chip layout: H=128 on partitions. Per step:
  - GRU gates via matmuls accumulating in PSUM (layer-0 input-side
    gates precomputed on host into gx0), gate math on DVE/ACT.
  - Attention scores e = tanh(Wh@hist + Ws@h_raw) recomputed on the
    tensor engine each step (PSUM accumulate fuses the q broadcast via
    a stride-0 rhs); tanh on ACT; logits via per-(layer,batch) flipped
    matmuls (e stationary, v moving) giving [t, lb] PSUM; softmax
    denominators via a ones-matmul; context via flipped matmuls over a
    transposed history copy (hist_T) maintained with a per-step
    PE-transpose + partition-collapse DMA.
"""

import sys

sys.path.insert(0, "/opt/trn_rl_repo")

import numpy as np

L = 2
B = 32
T = 128
H = 128
F = 3
D = 10
M = 8          # cores
BS = 4         # batch shards per stack
b = B // BS    # 8 rows per core
LB = L * b     # 16 (layer, batch) pairs per core

_FP32 = "float32"


def _build_program(mmdt_name: str, t_steps: int):
    """Build the per-core Bass program. Returns (nc, names dict)."""
    import concourse.bass as bass
    import concourse.bacc as bacc
    import concourse.mybir as mybir
    import concourse.tile as tile
    from concourse.masks import make_identity

    dt = mybir.dt
    AF = mybir.ActivationFunctionType
    ALU = mybir.AluOpType
    MMDT = getattr(dt, mmdt_name)
    P = 128

    nc = bacc.Bacc("TRN2", target_bir_lowering=False, debug=False)

    def din(name, shape, d=MMDT):
        return nc.dram_tensor(name, shape, d, kind="ExternalInput").ap()

    gx0_d = din("gx0", [P, t_steps, 3, b], dt.float32)
    whh0T_d = din("whh0T", [P, 3, H])
    whh1T_d = din("whh1T", [P, 3, H])
    wih1T_d = din("wih1T", [P, 3, H])
    brz1_d = din("brz1", [P, 2 * b], dt.float32)
    bn0h_d = din("bn0h", [P, 1], dt.float32)
    bn1h_d = din("bn1h", [P, 1], dt.float32)
    bn1i_d = din("bn1i", [P, 1], dt.float32)
    wsT_d = din("wsT", [P, H])
    whT_d = din("whT", [P, H])
    v_d = din("v", [P, 1])
    fc2cT_d = din("fc2cT", [P, H])
    fc2hT_d = din("fc2hT", [P, H])
    fc2b_d = din("fc2b", [P, 1], dt.float32)

    out_d = nc.dram_tensor("out", [P, t_steps, b], MMDT, kind="ExternalOutput").ap()

    with tile.TileContext(nc) as tc:
        with tc.tile_pool(name="const", bufs=1) as cp, \
             tc.tile_pool(name="state", bufs=1) as st, \
             tc.tile_pool(name="work", bufs=2) as wk, \
             tc.tile_pool(name="hbuf", bufs=2) as hb, \
             tc.tile_pool(name="ps_e", bufs=1, space="PSUM") as ps_e, \
             tc.tile_pool(name="ps_g", bufs=2, space="PSUM") as ps_g, \
             tc.tile_pool(name="ps_s", bufs=2, space="PSUM") as ps_s:

            # ---- constants / persistent state ----
            gx0 = cp.tile([P, t_steps, 3, b], dt.float32, tag="gx0")
            whh0T = cp.tile([P, 3, H], MMDT, tag="whh0T")
            whh1T = cp.tile([P, 3, H], MMDT, tag="whh1T")
            wih1T = cp.tile([P, 3, H], MMDT, tag="wih1T")
            brz1 = cp.tile([P, 2 * b], dt.float32, tag="brz1")
            bn0h = cp.tile([P, 1], dt.float32, tag="bn0h")
            bn1h = cp.tile([P, 1], dt.float32, tag="bn1h")
            bn1i = cp.tile([P, 1], dt.float32, tag="bn1i")
            wsT = cp.tile([P, H], MMDT, tag="wsT")
            whT = cp.tile([P, H], MMDT, tag="whT")
            vcol = cp.tile([P, 1], MMDT, tag="vcol")
            fc2cT = cp.tile([P, H], MMDT, tag="fc2cT")
            fc2hT = cp.tile([P, H], MMDT, tag="fc2hT")
            fc2b = cp.tile([P, 1], dt.float32, tag="fc2b")
            ones = cp.tile([P, P], dt.float32, tag="ones")
            ident = cp.tile([P, P], MMDT, tag="ident")
            h0t = cp.tile([P, LB], MMDT, tag="h0t")  # zero initial state

            for tl, d_ in [(gx0, gx0_d), (whh0T, whh0T_d), (whh1T, whh1T_d),
                           (wih1T, wih1T_d), (brz1, brz1_d), (bn0h, bn0h_d),
                           (bn1h, bn1h_d), (bn1i, bn1i_d), (wsT, wsT_d),
                           (whT, whT_d), (vcol, v_d), (fc2cT, fc2cT_d),
                           (fc2hT, fc2hT_d), (fc2b, fc2b_d)]:
                nc.sync.dma_start(tl[:], d_)
            nc.vector.memset(ones[:], 1.0)
            make_identity(nc, ident[:])
            nc.vector.memset(h0t[:], 0.0)

            hist_H = st.tile([P, t_steps, LB], MMDT, tag="hist_H")
            hist_T = st.tile([t_steps, LB * H], MMDT, tag="hist_T")
            e_sb = st.tile([P, t_steps, LB], MMDT, tag="e_sb")

            h_prev = h0t  # post-attention state from previous step

            for i in range(t_steps):
                ti = i + 1  # history length this step

                # ================= GRU (both layers) =================
                g_ps = ps_g.tile([P, 4 * b], dt.float32, tag="g")
                # L0: r, z, nh  (input-side gates precomputed in gx0)
                nc.tensor.matmul(g_ps[:, 0:b], whh0T[:, 0, :],
                                 h_prev[:, 0:b], start=True, stop=True)
                nc.tensor.matmul(g_ps[:, b:2 * b], whh0T[:, 1, :],
                                 h_prev[:, 0:b], start=True, stop=True)
                nc.tensor.matmul(g_ps[:, 3 * b:4 * b], whh0T[:, 2, :],
                                 h_prev[:, 0:b], start=True, stop=True)

                rzs0 = wk.tile([P, 2 * b], dt.float32, tag="rzs")
                nc.vector.scalar_tensor_tensor(
                    rzs0[:], g_ps[:, 0:2 * b], 1.0,
                    gx0[:, i, 0:2, :].rearrange("p a c -> p (a c)"),
                    op0=ALU.mult, op1=ALU.add)
                nc.scalar.activation(rzs0[:], rzs0[:], AF.Sigmoid)
                tmp0 = wk.tile([P, b], dt.float32, tag="tmp")
                nc.vector.scalar_tensor_tensor(
                    tmp0[:], g_ps[:, 3 * b:4 * b], bn0h[:, :], rzs0[:, 0:b],
                    op0=ALU.add, op1=ALU.mult)
                nin0 = wk.tile([P, b], dt.float32, tag="nin")
                nc.vector.scalar_tensor_tensor(
                    nin0[:], gx0[:, i, 2, :], 0.0, tmp0[:],
                    op0=ALU.add, op1=ALU.add)
                n0 = wk.tile([P, b], dt.float32, tag="nn")
                nc.scalar.activation(n0[:], nin0[:], AF.Tanh)
                d0 = wk.tile([P, b], dt.float32, tag="dd")
                nc.vector.scalar_tensor_tensor(
                    d0[:], h_prev[:, 0:b], 1.0, n0[:],
                    op0=ALU.mult, op1=ALU.subtract)
                zd0 = wk.tile([P, b], dt.float32, tag="zd")
                nc.vector.scalar_tensor_tensor(
                    zd0[:], rzs0[:, b:2 * b], 1.0, d0[:],
                    op0=ALU.mult, op1=ALU.mult)
                nc.vector.scalar_tensor_tensor(
                    hist_H[:, i, 0:b], zd0[:], 1.0, n0[:],
                    op0=ALU.mult, op1=ALU.add)

                # L1: input = h_raw0 = hist_H[:, i, 0:b]
                g1_ps = ps_g.tile([P, 4 * b], dt.float32, tag="g")
                h_raw0 = hist_H[:, i, 0:b]
                nc.tensor.matmul(g1_ps[:, 0:b], wih1T[:, 0, :], h_raw0,
                                 start=True, stop=False)
                nc.tensor.matmul(g1_ps[:, 0:b], whh1T[:, 0, :],
                                 h_prev[:, b:2 * b], start=False, stop=True)
                nc.tensor.matmul(g1_ps[:, b:2 * b], wih1T[:, 1, :], h_raw0,
                                 start=True, stop=False)
                nc.tensor.matmul(g1_ps[:, b:2 * b], whh1T[:, 1, :],
                                 h_prev[:, b:2 * b], start=False, stop=True)
                nc.tensor.matmul(g1_ps[:, 2 * b:3 * b], wih1T[:, 2, :], h_raw0,
                                 start=True, stop=True)
                nc.tensor.matmul(g1_ps[:, 3 * b:4 * b], whh1T[:, 2, :],
                                 h_prev[:, b:2 * b], start=True, stop=True)

                rzs1 = wk.tile([P, 2 * b], dt.float32, tag="rzs")
                nc.vector.scalar_tensor_tensor(
                    rzs1[:], g1_ps[:, 0:2 * b], 1.0, brz1[:],
                    op0=ALU.mult, op1=ALU.add)
                nc.scalar.activation(rzs1[:], rzs1[:], AF.Sigmoid)
                tmp1 = wk.tile([P, b], dt.float32, tag="tmp")
                nc.vector.scalar_tensor_tensor(
                    tmp1[:], g1_ps[:, 3 * b:4 * b], bn1h[:, :], rzs1[:, 0:b],
                    op0=ALU.add, op1=ALU.mult)
                nin1 = wk.tile([P, b], dt.float32, tag="nin")
                nc.vector.scalar_tensor_tensor(
                    nin1[:], g1_ps[:, 2 * b:3 * b], bn1i[:, :], tmp1[:],
                    op0=ALU.add, op1=ALU.add)
                n1 = wk.tile([P, b], dt.float32, tag="nn")
                nc.scalar.activation(n1[:], nin1[:], AF.Tanh)
                d1 = wk.tile([P, b], dt.float32, tag="dd")
                nc.vector.scalar_tensor_tensor(
                    d1[:], h_prev[:, b:2 * b], 1.0, n1[:],
                    op0=ALU.mult, op1=ALU.subtract)
                zd1 = wk.tile([P, b], dt.float32, tag="zd")
                nc.vector.scalar_tensor_tensor(
                    zd1[:], rzs1[:, b:2 * b], 1.0, d1[:],
                    op0=ALU.mult, op1=ALU.mult)
                nc.vector.scalar_tensor_tensor(
                    hist_H[:, i, b:2 * b], zd1[:], 1.0, n1[:],
                    op0=ALU.mult, op1=ALU.add)

                h_raw = hist_H[:, i, :]

                # ========== hist_T append (transpose + collapse DMA) ==========
                hT_ps = ps_s.tile([LB, H], MMDT, tag="sm")
                nc.tensor.transpose(hT_ps[:], h_raw, ident[:])
                trT = wk.tile([LB, H], MMDT, tag="trT")
                nc.vector.tensor_copy(trT[:], hT_ps[:])
                nc.sync.dma_start(
                    hist_T[i:i + 1, :].rearrange("p (a c) -> p a c", a=LB),
                    trT[:])

                if i == 0:
                    h_cur = hb.tile([P, LB], MMDT, tag="hcur")
                    nc.vector.tensor_copy(h_cur[:], h_raw)
                    h_prev = h_cur
                    continue

                # ================= attention =================
                e_ps = ps_e.tile([P, t_steps, LB], dt.float32, tag="e_ps")
                tchunk = 512 // LB  # 32 t-steps -> N=512 (one PSUM bank)
                for c0 in range(0, ti, tchunk):
                    nt = min(tchunk, ti - c0)
                    nc.tensor.matmul(e_ps[:, c0:c0 + nt, :], whT[:],
                                     hist_H[:, c0:c0 + nt, :],
                                     start=True, stop=False)
                    nc.tensor.matmul(
                        e_ps[:, c0:c0 + nt, :], wsT[:],
                        hist_H[:, i:i + 1, :].broadcast_to([P, nt, LB]),
                        start=False, stop=True)
                nc.scalar.activation(e_sb[:, 0:ti, :], e_ps[:, 0:ti, :], AF.Tanh)

                log_ps = ps_s.tile([P, LB], dt.float32, tag="sm")
                for lb in range(LB):
                    nc.tensor.matmul(log_ps[0:ti, lb:lb + 1],
                                     e_sb[:, 0:ti, lb], vcol[:],
                                     start=True, stop=True)
                u_sb = wk.tile([P, LB], dt.float32, tag="u")
                nc.scalar.activation(u_sb[0:ti, :], log_ps[0:ti, :], AF.Exp)

                den_ps = ps_s.tile([1, LB], dt.float32, tag="sm")
                nc.tensor.matmul(den_ps[:], ones[0:ti, 0:1], u_sb[0:ti, :],
                                 start=True, stop=True)
                rden = wk.tile([1, LB], dt.float32, tag="rden")
                nc.vector.reciprocal(rden[:], den_ps[:])
                rbc_ps = ps_s.tile([P, LB], dt.float32, tag="sm")
                nc.tensor.matmul(rbc_ps[0:ti, :], ones[0:1, 0:ti], rden[:],
                                 start=True, stop=True)
                aT = wk.tile([P, LB], MMDT, tag="aT")
                nc.vector.tensor_mul(aT[0:ti, :], u_sb[0:ti, :], rbc_ps[0:ti, :])

                c_ps = ps_s.tile([P, LB], dt.float32, tag="sm")
                for lb in range(LB):
                    nc.tensor.matmul(c_ps[:, lb:lb + 1],
                                     hist_T[0:ti, lb * H:(lb + 1) * H],
                                     aT[0:ti, lb:lb + 1],
                                     start=True, stop=True)
                c_sb = wk.tile([P, LB], MMDT, tag="c_sb")
                nc.vector.tensor_copy(c_sb[:], c_ps[:])

                att_ps = ps_s.tile([P, LB], dt.float32, tag="sm")
                nc.tensor.matmul(att_ps[:], fc2cT[:], c_sb[:],
                                 start=True, stop=False)
                nc.tensor.matmul(att_ps[:], fc2hT[:], h_raw,
                                 start=False, stop=True)
                h_cur = hb.tile([P, LB], MMDT, tag="hcur")
                nc.scalar.activation(h_cur[:], att_ps[:], AF.Identity,
                                     bias=fc2b[:, :])
                h_prev = h_cur

            # top-layer raw outputs
            nc.sync.dma_start(out_d, hist_H[:, :, b:2 * b])

    nc.compile()
    return nc


def _prep_core_inputs(inputs, s, q, mmdt, t_steps):
    """Host-side weight/input transforms for core (stack s in {1,2}, quarter q)."""
    f32 = lambda a: np.asarray(a, dtype=np.float32)
    rows = slice(q * b, (q + 1) * b)
    x = f32(inputs["received"])[rows, :t_steps]          # [b, t, F]
    Wih0, Whh0 = f32(inputs[f"Wih{s}_0"]), f32(inputs[f"Whh{s}_0"])
    bih0, bhh0 = f32(inputs[f"bih{s}_0"]), f32(inputs[f"bhh{s}_0"])
    Wih1, Whh1 = f32(inputs[f"Wih{s}_1"]), f32(inputs[f"Whh{s}_1"])
    bih1, bhh1 = f32(inputs[f"bih{s}_1"]), f32(inputs[f"bhh{s}_1"])
    attn_W, v_W = f32(inputs["attn_W"]), f32(inputs["v_W"])
    fc2_W, fc2_b = f32(inputs["fc2_W"]), f32(inputs["fc2_b"])

    gx0 = x @ Wih0.T + bih0                               # [b, t, 384]
    gx0[:, :, 0:2 * H] += bhh0[0:2 * H]                   # fold bhh into r,z
    # device layout [H, t, g, b]
    gx0_dev = np.ascontiguousarray(
        gx0.reshape(b, t_steps, 3, H).transpose(3, 1, 2, 0)).astype(np.float32)

    def wT3(W):  # [3H, Hin] -> [Hin, 3, Hout]
        return np.ascontiguousarray(
            W.reshape(3, H, -1).transpose(2, 0, 1)).astype(mmdt)

    brz1 = np.empty((H, 2 * b), np.float32)
    brz1[:, 0:b] = (bih1[0:H] + bhh1[0:H])[:, None]
    brz1[:, b:2 * b] = (bih1[H:2 * H] + bhh1[H:2 * H])[:, None]

    return {
        "gx0": gx0_dev,
        "whh0T": wT3(Whh0), "whh1T": wT3(Whh1), "wih1T": wT3(Wih1),
        "brz1": brz1,
        "bn0h": bhh0[2 * H:, None].astype(np.float32),
        "bn1h": bhh1[2 * H:, None].astype(np.float32),
        "bn1i": bih1[2 * H:, None].astype(np.float32),
        "wsT": np.ascontiguousarray(attn_W[:, :H].T).astype(mmdt),
        "whT": np.ascontiguousarray(attn_W[:, H:].T).astype(mmdt),
        "v": v_W[0][:, None].astype(mmdt),
        "fc2cT": np.ascontiguousarray(fc2_W[:, :H].T).astype(mmdt),
        "fc2hT": np.ascontiguousarray(fc2_W[:, H:].T).astype(mmdt),
        "fc2b": fc2_b[:, None].astype(np.float32),
    }


_CACHE = {}


def _get_program(mmdt_name, t_steps):
    key = (mmdt_name, t_steps)
    if key not in _CACHE:
        _CACHE[key] = _build_program(mmdt_name, t_steps)
    return _CACHE[key]


def _run_on_device(inputs, mmdt_name="float32", t_steps=T, trace=False):
    from concourse.bass_utils import run_bass_kernel_spmd

    mmdt = np.float32 if mmdt_name == "float32" else np.dtype("bfloat16")
    try:
        np_mmdt = np.dtype(mmdt_name)
    except TypeError:
        import ml_dtypes
        np_mmdt = np.dtype(ml_dtypes.bfloat16)
    if mmdt_name != "float32":
        import ml_dtypes
        np_mmdt = np.dtype(ml_dtypes.bfloat16)
    else:
        np_mmdt = np.dtype(np.float32)

    nc = _get_program(mmdt_name, t_steps)
    in_maps = []
    for c in range(M):
        s, q = c // BS + 1, c % BS
        in_maps.append(_prep_core_inputs(inputs, s, q, np_mmdt, t_steps))
    res = run_bass_kernel_spmd(nc, in_maps, list(range(M)), trace=trace)
    outs = [np.asarray(r["out"], dtype=np.float32) for r in res.results]
    return outs, res


def _finish_host(inputs, outs, t_steps=T):
    """outs: per-core [H, b, t] top-layer raw states -> final [B, T, 1]."""
    f32 = lambda a: np.asarray(a, dtype=np.float32)
    rnn = np.empty((2, B, t_steps, H), np.float32)
    for c in range(M):
        s, q = c // BS, c % BS
        rnn[s, q * b:(q + 1) * b] = outs[c].transpose(2, 1, 0)
    idx = np.minimum(np.arange(t_steps) + D, t_steps - 1)
    rt_d = rnn[1][:, idx, :]
    out_W, out_b = f32(inputs["out_W"]), f32(inputs["out_b"])
    dec = np.tanh(
        rnn[0] @ out_W[:, :H].T + rt_d @ out_W[:, H:].T + out_b)
    return (1.0 / (1.0 + np.exp(-dec))).astype(np.float32)


MMDT_NAME = "float32"


def kernel(received,
           Wih1_0, Whh1_0, bih1_0, bhh1_0, Wih1_1, Whh1_1, bih1_1, bhh1_1,
           Wih2_0, Whh2_0, bih2_0, bhh2_0, Wih2_1, Whh2_1, bih2_1, bhh2_1,
           attn_W, v_W, fc2_W, fc2_b, out_W, out_b):
    inputs = dict(
        received=received,
        Wih1_0=Wih1_0, Whh1_0=Whh1_0, bih1_0=bih1_0, bhh1_0=bhh1_0,
        Wih1_1=Wih1_1, Whh1_1=Whh1_1, bih1_1=bih1_1, bhh1_1=bhh1_1,
        Wih2_0=Wih2_0, Whh2_0=Whh2_0, bih2_0=bih2_0, bhh2_0=bhh2_0,
        Wih2_1=Wih2_1, Whh2_1=Whh2_1, bih2_1=bih2_1, bhh2_1=bhh2_1,
        attn_W=attn_W, v_W=v_W, fc2_W=fc2_W, fc2_b=fc2_b,
        out_W=out_W, out_b=out_b)
    outs, _ = _run_on_device(inputs, MMDT_NAME, T)
    return _finish_host(inputs, outs, T)
